# revision 8
# baseline (speedup 1.0000x reference)
"""HEX loss kernel for Trainium2 (8 NeuronCores, batch-parallel, raw Bass).

Math: the chain junction-tree potential is rank-1 per clique and each
interior fs[v] is split fs[v]/2 over its two cliques, so the joint
distribution factorizes into independent Bernoullis with
P(y_v=1) = sigmoid(fs[b,v]); hence
    loss = mean_b softplus(-fs[b, labels[b]])
(verified to 1.4e-16 vs the f64 junction-tree reference).

Per core (4096 rows, pure data parallel): stream fs (4 MB) as 4x1MB
SWDGE cast-DMAs (f32->bf16). Exact gather via max-trick: ACT computes
penalty = Square(10*iota - 10*lab) per row-tile, DVE does one wide
[128,2048] subtract per group and a grouped reduce_max ->
sel = fs[b, lab]. softplus(-x) = ln(1+exp(-x)) is built from Exp only
(gen3 ACT tables have no Ln): exponent-bit log2 initial guess, then 2
Newton steps y += z*exp(-y) - 1. Host sums 8x128 partials / B.
"""

import numpy as np

B = 32768
V = 256
N_CORES = 8
BL = B // N_CORES
P = 128
RPP = 8
GROUP_ROWS = P * RPP       # 1024 rows, 1 MB f32
N_GROUPS = BL // GROUP_ROWS  # 4
NT = BL // P               # 32
N_SQRT = 8
LOG_SCALE = float(1 << N_SQRT)
N_NEWTON = 2
PEN = 10.0

_CACHE = {}


def _build():
    from contextlib import ExitStack

    import concourse.bass as bass
    import concourse.tile as tile  # noqa
    from concourse import bacc, mybir

    f32 = mybir.dt.float32
    bf16 = mybir.dt.bfloat16
    Alu = mybir.AluOpType
    Act = mybir.ActivationFunctionType

    nc = bacc.Bacc(
        "TRN2",
        target_bir_lowering=False,
        debug=False,
        enable_asserts=True,
        num_devices=N_CORES,
    )

    fs_d = nc.dram_tensor("fs", [BL, V], f32, kind="ExternalInput").ap()
    lab_d = nc.dram_tensor("labt", [P, NT], f32, kind="ExternalInput").ap()
    out_d = nc.dram_tensor("out", [P, 1], f32, kind="ExternalOutput").ap()

    fs_view = fs_d.rearrange("(g p j) v -> g p (j v)", g=N_GROUPS, p=P, j=RPP)

    with ExitStack() as ctx:
        # SBUF allocations
        iota = ctx.enter_context(nc.sbuf_tensor([P, V], f32))
        labt = ctx.enter_context(nc.sbuf_tensor([P, NT], f32))
        labp = ctx.enter_context(nc.sbuf_tensor([P, NT], f32))
        sel = ctx.enter_context(nc.sbuf_tensor([P, NT], f32))
        fs_t = [ctx.enter_context(nc.sbuf_tensor(f"fs_t{i}", [P, RPP * V], bf16)) for i in range(N_GROUPS)]
        sq_big = [ctx.enter_context(nc.sbuf_tensor(f"sq_big{i}", [P, RPP * V], bf16)) for i in range(2)]
        prod = [ctx.enter_context(nc.sbuf_tensor(f"prod{i}", [P, RPP * V], bf16)) for i in range(2)]
        # epilogue tiles
        u = ctx.enter_context(nc.sbuf_tensor([P, NT], f32))
        z = ctx.enter_context(nc.sbuf_tensor([P, NT], f32))
        w1 = ctx.enter_context(nc.sbuf_tensor([P, NT], f32))
        w2 = ctx.enter_context(nc.sbuf_tensor([P, NT], f32))
        sS = ctx.enter_context(nc.sbuf_tensor([P, NT], f32))
        tA = ctx.enter_context(nc.sbuf_tensor([P, NT], f32))
        tB = ctx.enter_context(nc.sbuf_tensor([P, NT], f32))
        yv = ctx.enter_context(nc.sbuf_tensor([P, NT], f32))
        ey = ctx.enter_context(nc.sbuf_tensor([P, NT], f32))
        jnk = ctx.enter_context(nc.sbuf_tensor([P, NT], f32))
        acc = ctx.enter_context(nc.sbuf_tensor([P, 1], f32))

        sem_iota = ctx.enter_context(nc.semaphore("s_iota"))
        sem_lab = ctx.enter_context(nc.semaphore("s_lab"))
        sem_labp = ctx.enter_context(nc.semaphore("s_labp"))
        sem_fs = [ctx.enter_context(nc.semaphore(f"s_fs{g}")) for g in range(N_GROUPS)]
        sem_fs0b = ctx.enter_context(nc.semaphore("s_fs0b"))
        sem_sq = ctx.enter_context(nc.semaphore("s_sq"))
        sem_sub = ctx.enter_context(nc.semaphore("s_sub"))
        sem_red = ctx.enter_context(nc.semaphore("s_red"))
        sem_epa = ctx.enter_context(nc.semaphore("s_epa"))  # ACT -> DVE
        sem_epd = ctx.enter_context(nc.semaphore("s_epd"))  # DVE -> ACT
        sem_acc = ctx.enter_context(nc.semaphore("s_acc"))
        sem_out = ctx.enter_context(nc.semaphore("s_out"))

        blk = ctx.enter_context(nc.Block())

        @blk.gpsimd
        def _(g_eng):
            g_eng.iota(
                iota.ap(),
                pattern=[[1, V]],
                base=0,
                channel_multiplier=0,
                allow_small_or_imprecise_dtypes=True,
            ).then_inc(sem_iota, 1)
            HF = RPP * V // 2
            g_eng.dma_start(
                out=fs_t[0].ap()[:, :HF], in_=fs_view[0][:, :HF]
            ).then_inc(sem_fs[0], 16)
            g_eng.dma_start(
                out=fs_t[0].ap()[:, HF:], in_=fs_view[0][:, HF:]
            ).then_inc(sem_fs0b, 16)
            for g in range(1, N_GROUPS):
                g_eng.dma_start(out=fs_t[g].ap(), in_=fs_view[g]).then_inc(
                    sem_fs[g], 16
                )

        @blk.sync
        def _(s_eng):
            s_eng.dma_start(out=labt.ap(), in_=lab_d).then_inc(sem_lab, 16)
            s_eng.wait_ge(sem_acc, 1)
            s_eng.dma_start(out=out_d, in_=acc.ap()).then_inc(sem_out, 16)
            s_eng.wait_ge(sem_out, 16)

        @blk.scalar
        def _(a_eng):
            a_eng.wait_ge(sem_iota, 1)
            a_eng.wait_ge(sem_labp, 1)
            for t in range(NT):
                g, j = t // RPP, t % RPP
                if j == 0 and g >= 2:
                    a_eng.wait_ge(sem_sub, g - 1)
                a_eng.activation(
                    sq_big[g % 2].ap()[:, j * V : (j + 1) * V],
                    iota.ap(),
                    Act.Square,
                    scale=PEN,
                    bias=labp.ap()[:, t : t + 1],
                ).then_inc(sem_sq, 1)
            # epilogue (ACT side)
            a_eng.wait_ge(sem_red, N_GROUPS + 1)
            a_eng.activation(u.ap(), sel.ap(), Act.Exp, scale=-1.0).then_inc(
                sem_epa, 1
            )
            for i in range(N_NEWTON):
                a_eng.wait_ge(sem_epd, i + 1)
                a_eng.activation(ey.ap(), yv.ap(), Act.Exp, scale=-1.0).then_inc(
                    sem_epa, 1
                )

        @blk.vector
        def _(v_eng):
            v_eng.wait_ge(sem_lab, 16)
            v_eng.tensor_scalar(labp.ap(), labt.ap(), -PEN, None, Alu.mult).then_inc(
                sem_labp, 1
            )
            HF = RPP * V // 2
            HJ = RPP // 2
            pr0 = prod[0]
            v_eng.wait_ge(sem_fs[0], 16)
            v_eng.wait_ge(sem_sq, HJ)
            v_eng.tensor_sub(
                pr0.ap()[:, :HF], fs_t[0].ap()[:, :HF], sq_big[0].ap()[:, :HF]
            )
            v_eng.drain()
            v_eng.tensor_reduce(
                sel.ap()[:, 0:HJ],
                pr0.ap()[:, :HF].rearrange("p (j v) -> p j v", j=HJ),
                axis=mybir.AxisListType.X,
                op=Alu.max,
            ).then_inc(sem_red, 1)
            v_eng.wait_ge(sem_fs0b, 16)
            v_eng.wait_ge(sem_sq, RPP)
            v_eng.tensor_sub(
                pr0.ap()[:, HF:], fs_t[0].ap()[:, HF:], sq_big[0].ap()[:, HF:]
            ).then_inc(sem_sub, 1)
            v_eng.drain()
            v_eng.tensor_reduce(
                sel.ap()[:, HJ:RPP],
                pr0.ap()[:, HF:].rearrange("p (j v) -> p j v", j=HJ),
                axis=mybir.AxisListType.X,
                op=Alu.max,
            ).then_inc(sem_red, 1)
            for g in range(1, N_GROUPS):
                v_eng.wait_ge(sem_fs[g], 16)
                v_eng.wait_ge(sem_sq, RPP * (g + 1))
                pr = prod[g % 2]
                v_eng.tensor_sub(
                    pr.ap(), fs_t[g].ap(), sq_big[g % 2].ap()
                ).then_inc(sem_sub, 1)
                v_eng.drain()
                v_eng.tensor_reduce(
                    sel.ap()[:, g * RPP : (g + 1) * RPP],
                    pr.ap().rearrange("p (j v) -> p j v", j=RPP),
                    axis=mybir.AxisListType.X,
                    op=Alu.max,
                ).then_inc(sem_red, 1)
            # epilogue (DVE side)
            v_eng.wait_ge(sem_epa, 1)
            v_eng.tensor_scalar(z.ap(), u.ap(), 1.0, None, Alu.add)
            v_eng.drain()
            # y0 = ln2 * (float(bitcast_i32(z)) / 2^23 - 127): log2 from the
            # exponent+mantissa bits, max abs err ~0.06 -- Newton polishes.
            v_eng.tensor_copy(tA.ap(), z.ap().bitcast(mybir.dt.int32))
            v_eng.drain()
            v_eng.tensor_scalar(
                yv.ap(), tA.ap(), 0.6931471805599453 / (1 << 23),
                -127.0 * 0.6931471805599453, Alu.mult, Alu.add,
            ).then_inc(sem_epd, 1)
            for i in range(N_NEWTON):
                v_eng.wait_ge(sem_epa, 2 + i)
                v_eng.tensor_mul(tB.ap(), z.ap(), ey.ap())
                v_eng.drain()
                v_eng.tensor_add(tA.ap(), yv.ap(), tB.ap())
                v_eng.drain()
                v_eng.tensor_scalar(yv.ap(), tA.ap(), -1.0, None, Alu.add).then_inc(
                    sem_epd, 1
                )
            v_eng.drain()
            v_eng.tensor_reduce(
                acc.ap(), yv.ap(), axis=mybir.AxisListType.X, op=Alu.add
            ).then_inc(sem_acc, 1)

    nc.compile()
    return nc


def _get_nc():
    if "nc" not in _CACHE:
        _CACHE["nc"] = _build()
    return _CACHE["nc"]


def _shard_inputs(fs, labels):
    fs = np.ascontiguousarray(np.asarray(fs, dtype=np.float32))
    labels = np.asarray(labels)
    in_maps = []
    for c in range(N_CORES):
        fs_loc = fs[c * BL : (c + 1) * BL]
        lab_loc = labels[c * BL : (c + 1) * BL]
        labt = (
            lab_loc.reshape(N_GROUPS, P, RPP)
            .transpose(1, 0, 2)
            .reshape(P, NT)
            .astype(np.float32)
        )
        in_maps.append({"fs": fs_loc, "labt": np.ascontiguousarray(labt)})
    return in_maps


def kernel(fs, labels, _trace=False, _trace_kwargs=None):
    from concourse.bass_utils import run_bass_kernel_spmd

    nc = _get_nc()
    in_maps = _shard_inputs(fs, labels)
    res = run_bass_kernel_spmd(
        nc,
        in_maps,
        core_ids=list(range(N_CORES)),
        trace=_trace,
        **(_trace_kwargs or {}),
    )
    total = np.float64(0.0)
    for c in range(N_CORES):
        total += res.results[c]["out"].astype(np.float64).sum()
    loss = total / np.float64(B)
    if _trace:
        return np.float64(loss), res
    return np.asarray(loss, dtype=np.float64)


# revision 9
# speedup vs baseline: 1.0033x; 1.0033x over previous
"""HEX loss kernel for Trainium2 (8 NeuronCores, batch-parallel, raw Bass).

Math: the chain junction-tree potential is rank-1 per clique and each
interior fs[v] is split fs[v]/2 over its two cliques, so the joint
distribution factorizes into independent Bernoullis with
P(y_v=1) = sigmoid(fs[b,v]); hence
    loss = mean_b softplus(-fs[b, labels[b]])
(verified to 1.4e-16 vs the f64 junction-tree reference).

Per core (4096 rows, pure data parallel): stream fs (4 MB) as 4x1MB
SWDGE cast-DMAs (f32->bf16). Exact gather via max-trick: ACT computes
penalty = Square(10*iota - 10*lab) per row-tile, DVE does one wide
[128,2048] subtract per group and a grouped reduce_max ->
sel = fs[b, lab]. softplus(-x) = ln(1+exp(-x)) is built from Exp only
(gen3 ACT tables have no Ln): exponent-bit log2 initial guess, then 2
Newton steps y += z*exp(-y) - 1. Host sums 8x128 partials / B.
"""

import numpy as np

B = 32768
V = 256
N_CORES = 8
BL = B // N_CORES
P = 128
RPP = 8
GROUP_ROWS = P * RPP       # 1024 rows, 1 MB f32
N_GROUPS = BL // GROUP_ROWS  # 4
NT = BL // P               # 32
N_SQRT = 8
LOG_SCALE = float(1 << N_SQRT)
N_NEWTON = 2
PEN = 10.0

_CACHE = {}


def _build():
    from contextlib import ExitStack

    import concourse.bass as bass
    import concourse.tile as tile  # noqa
    from concourse import bacc, mybir

    f32 = mybir.dt.float32
    bf16 = mybir.dt.bfloat16
    Alu = mybir.AluOpType
    Act = mybir.ActivationFunctionType

    nc = bacc.Bacc(
        "TRN2",
        target_bir_lowering=False,
        debug=False,
        enable_asserts=True,
        num_devices=N_CORES,
    )

    fs_d = nc.dram_tensor("fs", [BL, V], f32, kind="ExternalInput").ap()
    lab_d = nc.dram_tensor("labt", [P, NT], f32, kind="ExternalInput").ap()
    out_d = nc.dram_tensor("out", [P, 1], f32, kind="ExternalOutput").ap()

    fs_view = fs_d.rearrange("(g p j) v -> g p (j v)", g=N_GROUPS, p=P, j=RPP)

    with ExitStack() as ctx:
        # SBUF allocations
        iota = ctx.enter_context(nc.sbuf_tensor([P, V], f32))
        labt = ctx.enter_context(nc.sbuf_tensor([P, NT], f32))
        labp = ctx.enter_context(nc.sbuf_tensor([P, NT], f32))
        sel = ctx.enter_context(nc.sbuf_tensor([P, NT], f32))
        fs_t = [ctx.enter_context(nc.sbuf_tensor(f"fs_t{i}", [P, RPP * V], bf16)) for i in range(N_GROUPS)]
        sq_big = [ctx.enter_context(nc.sbuf_tensor(f"sq_big{i}", [P, RPP * V], bf16)) for i in range(2)]
        prod = [ctx.enter_context(nc.sbuf_tensor(f"prod{i}", [P, RPP * V], bf16)) for i in range(2)]
        # epilogue tiles
        u = ctx.enter_context(nc.sbuf_tensor([P, NT], f32))
        z = ctx.enter_context(nc.sbuf_tensor([P, NT], f32))
        w1 = ctx.enter_context(nc.sbuf_tensor([P, NT], f32))
        w2 = ctx.enter_context(nc.sbuf_tensor([P, NT], f32))
        sS = ctx.enter_context(nc.sbuf_tensor([P, NT], f32))
        tA = ctx.enter_context(nc.sbuf_tensor([P, NT], f32))
        tB = ctx.enter_context(nc.sbuf_tensor([P, NT], f32))
        yv = ctx.enter_context(nc.sbuf_tensor([P, NT], f32))
        ey = ctx.enter_context(nc.sbuf_tensor([P, NT], f32))
        jnk = ctx.enter_context(nc.sbuf_tensor([P, NT], f32))
        acc = ctx.enter_context(nc.sbuf_tensor([P, 1], f32))

        sem_iota = ctx.enter_context(nc.semaphore("s_iota"))
        sem_lab = ctx.enter_context(nc.semaphore("s_lab"))
        sem_labp = ctx.enter_context(nc.semaphore("s_labp"))
        sem_fs = [ctx.enter_context(nc.semaphore(f"s_fs{g}")) for g in range(N_GROUPS)]
        sem_sq = ctx.enter_context(nc.semaphore("s_sq"))
        sem_sub = ctx.enter_context(nc.semaphore("s_sub"))
        sem_red = ctx.enter_context(nc.semaphore("s_red"))
        sem_epa = ctx.enter_context(nc.semaphore("s_epa"))  # ACT -> DVE
        sem_epd = ctx.enter_context(nc.semaphore("s_epd"))  # DVE -> ACT
        sem_acc = ctx.enter_context(nc.semaphore("s_acc"))
        sem_out = ctx.enter_context(nc.semaphore("s_out"))

        blk = ctx.enter_context(nc.Block())

        @blk.gpsimd
        def _(g_eng):
            g_eng.iota(
                iota.ap(),
                pattern=[[1, V]],
                base=0,
                channel_multiplier=0,
                allow_small_or_imprecise_dtypes=True,
            ).then_inc(sem_iota, 1)
            for g in range(N_GROUPS):
                g_eng.dma_start(out=fs_t[g].ap(), in_=fs_view[g]).then_inc(
                    sem_fs[g], 16
                )

        @blk.sync
        def _(s_eng):
            s_eng.dma_start(out=labt.ap(), in_=lab_d).then_inc(sem_lab, 16)
            s_eng.wait_ge(sem_acc, 1)
            s_eng.dma_start(out=out_d, in_=acc.ap()).then_inc(sem_out, 16)
            s_eng.wait_ge(sem_out, 16)

        @blk.scalar
        def _(a_eng):
            a_eng.wait_ge(sem_iota, 1)
            a_eng.wait_ge(sem_labp, 1)
            for t in range(NT):
                g, j = t // RPP, t % RPP
                if j == 0 and g >= 2:
                    a_eng.wait_ge(sem_sub, g - 1)
                a_eng.activation(
                    sq_big[g % 2].ap()[:, j * V : (j + 1) * V],
                    iota.ap(),
                    Act.Square,
                    scale=PEN,
                    bias=labp.ap()[:, t : t + 1],
                ).then_inc(sem_sq, 1)
            # epilogue (ACT side)
            a_eng.wait_ge(sem_red, N_GROUPS)
            a_eng.activation(u.ap(), sel.ap(), Act.Exp, scale=-1.0).then_inc(
                sem_epa, 1
            )
            for i in range(N_NEWTON):
                a_eng.wait_ge(sem_epd, i + 1)
                a_eng.activation(ey.ap(), yv.ap(), Act.Exp, scale=-1.0).then_inc(
                    sem_epa, 1
                )

        @blk.vector
        def _(v_eng):
            v_eng.wait_ge(sem_lab, 16)
            v_eng.tensor_scalar(labp.ap(), labt.ap(), -PEN, None, Alu.mult).then_inc(
                sem_labp, 1
            )
            for g in range(N_GROUPS):
                v_eng.wait_ge(sem_fs[g], 16)
                v_eng.wait_ge(sem_sq, RPP * (g + 1))
                pr = prod[g % 2]
                v_eng.tensor_sub(
                    pr.ap(), fs_t[g].ap(), sq_big[g % 2].ap()
                ).then_inc(sem_sub, 1)
                v_eng.drain()
                v_eng.tensor_reduce(
                    sel.ap()[:, g * RPP : (g + 1) * RPP],
                    pr.ap().rearrange("p (j v) -> p j v", j=RPP),
                    axis=mybir.AxisListType.X,
                    op=Alu.max,
                ).then_inc(sem_red, 1)
            # epilogue (DVE side)
            v_eng.wait_ge(sem_epa, 1)
            v_eng.tensor_scalar(z.ap(), u.ap(), 1.0, None, Alu.add)
            v_eng.drain()
            # y0 = ln2 * (float(bitcast_i32(z)) / 2^23 - 127): log2 from the
            # exponent+mantissa bits, max abs err ~0.06 -- Newton polishes.
            v_eng.tensor_copy(tA.ap(), z.ap().bitcast(mybir.dt.int32))
            v_eng.drain()
            v_eng.tensor_scalar(
                yv.ap(), tA.ap(), 0.6931471805599453 / (1 << 23),
                -127.0 * 0.6931471805599453, Alu.mult, Alu.add,
            ).then_inc(sem_epd, 1)
            for i in range(N_NEWTON):
                v_eng.wait_ge(sem_epa, 2 + i)
                v_eng.tensor_mul(tB.ap(), z.ap(), ey.ap())
                v_eng.drain()
                v_eng.tensor_add(tA.ap(), yv.ap(), tB.ap())
                v_eng.drain()
                v_eng.tensor_scalar(yv.ap(), tA.ap(), -1.0, None, Alu.add).then_inc(
                    sem_epd, 1
                )
            v_eng.drain()
            v_eng.tensor_reduce(
                acc.ap(), yv.ap(), axis=mybir.AxisListType.X, op=Alu.add
            ).then_inc(sem_acc, 1)

    nc.compile()
    return nc


def _get_nc():
    if "nc" not in _CACHE:
        _CACHE["nc"] = _build()
    return _CACHE["nc"]


def _shard_inputs(fs, labels):
    fs = np.ascontiguousarray(np.asarray(fs, dtype=np.float32))
    labels = np.asarray(labels)
    in_maps = []
    for c in range(N_CORES):
        fs_loc = fs[c * BL : (c + 1) * BL]
        lab_loc = labels[c * BL : (c + 1) * BL]
        labt = (
            lab_loc.reshape(N_GROUPS, P, RPP)
            .transpose(1, 0, 2)
            .reshape(P, NT)
            .astype(np.float32)
        )
        in_maps.append({"fs": fs_loc, "labt": np.ascontiguousarray(labt)})
    return in_maps


def kernel(fs, labels, _trace=False, _trace_kwargs=None):
    from concourse.bass_utils import run_bass_kernel_spmd

    nc = _get_nc()
    in_maps = _shard_inputs(fs, labels)
    res = run_bass_kernel_spmd(
        nc,
        in_maps,
        core_ids=list(range(N_CORES)),
        trace=_trace,
        **(_trace_kwargs or {}),
    )
    total = np.float64(0.0)
    for c in range(N_CORES):
        total += res.results[c]["out"].astype(np.float64).sum()
    loss = total / np.float64(B)
    if _trace:
        return np.float64(loss), res
    return np.asarray(loss, dtype=np.float64)
